# revision 1
# baseline (speedup 1.0000x reference)
"""Trainium2 Bass kernel for a 3-layer conditional LSTM (SMILES RNN) with
encoder/decoder feedback.

Math reformulation (verified vs the jax reference):
  - The decoder->encoder feedback path is folded through the rank-47 logits:
      gates0 = A0 @ logits_prev + Wp0 @ props + Whh0 @ h0 + b0c
    with A0 = w_ih0[:, :H] @ enc_w, Wp0 = w_ih0[:, H:], and
    b0c = w_ih0[:, :H] @ enc_b + b_ih0 + b_hh0.  [A0 | Wp0 | b0c] forms one
    K=52 augmented contraction whose stationary operand is
    [logits.T; props.T; ones].
  - t=0 is uniform with logits_init = onehot(1) (the start token encodes to
    exactly enc_w @ onehot1 + enc_b).
  - Logits are produced per-step into an SBUF history buffer and DMA'd out
    once at the end.

Distribution: pure data parallel, batch 128 -> 16 rows per core, weights
replicated; the sequential scan stays core-local (no collectives).

Layout: activations batch-on-partition [16, *]; weights are the *moving*
matmul operand streamed as float32r (full fp32 storage, ~1e-4 matmul
accuracy, 1 cycle/row on TRN2 for moving dim >= 256).  The per-step h must
be transposed ([16,512] -> 4x [128,16]) to serve as the next stationary
operand; done on the PE with an identity matmul.
"""

import numpy as np

B, T, H, O, P, NL = 128, 64, 512, 47, 4, 3
G = 4 * H
NCORES = 8
BL = B // NCORES
KAUG = O + P + 1  # 52
OP = 48  # O padded to even width (fp32r ISA: innermost free count must be even)
MM_DT = "float16"  # matmul operand dtype: "float16" or "float32r"


def _build_nc(t_steps):
    import concourse.mybir as mybir
    import concourse.tile as tile
    from concourse import bacc
    from concourse.masks import make_identity

    F32 = mybir.dt.float32
    F32R = getattr(mybir.dt, MM_DT)
    ACT = mybir.ActivationFunctionType

    nc = bacc.Bacc(None, target_bir_lowering=False)

    w0aug_d = nc.dram_tensor("w0aug", [KAUG, G], F32R, kind="ExternalInput")
    whh0_d = nc.dram_tensor("whh0", [128, 4, G], F32R, kind="ExternalInput")
    w1_d = nc.dram_tensor("w1", [128, 8, G], F32R, kind="ExternalInput")
    w2_d = nc.dram_tensor("w2", [128, 8, G], F32R, kind="ExternalInput")
    dec_d = nc.dram_tensor("dec", [128, 4, OP], F32R, kind="ExternalInput")
    b1_d = nc.dram_tensor("b1", [1, G], F32R, kind="ExternalInput")
    b2_d = nc.dram_tensor("b2", [1, G], F32R, kind="ExternalInput")
    decb_d = nc.dram_tensor("dec_b", [1, OP], F32R, kind="ExternalInput")
    xaug_d = nc.dram_tensor("xaug0", [KAUG, BL], F32R, kind="ExternalInput")
    init_d = nc.dram_tensor("init", [128, NL * 4 * BL + BL], F32R, kind="ExternalInput")
    out_d = nc.dram_tensor("out", [BL, t_steps * O], F32, kind="ExternalOutput")

    with tile.TileContext(nc) as tc:
        with (
            tc.tile_pool(name="weights", bufs=1) as wp,
            tc.tile_pool(name="state", bufs=1) as sp,
            tc.tile_pool(name="htmp", bufs=1) as hp,
            tc.tile_pool(name="gpool", bufs=6 if globals().get("_NCH", 4) == 4 else 3, space="PSUM") as gp,
            tc.tile_pool(name="tpool", bufs=2, space="PSUM") as tp,
        ):
            w0aug = wp.tile([KAUG, G], F32R)
            nc.gpsimd.dma_start(w0aug[:], w0aug_d[:])
            whh0 = wp.tile([128, 4, G], F32R)
            nc.gpsimd.dma_start(whh0[:], whh0_d[:])
            w1 = wp.tile([128, 8, G], F32R)
            nc.gpsimd.dma_start(w1[:], w1_d[:])
            w2 = wp.tile([128, 8, G], F32R)
            nc.gpsimd.dma_start(w2[:], w2_d[:])
            dec = wp.tile([128, 4, OP], F32R)
            nc.gpsimd.dma_start(dec[:], dec_d[:])
            b1 = wp.tile([1, G], F32R)
            nc.gpsimd.dma_start(b1[:], b1_d[:])
            b2 = wp.tile([1, G], F32R)
            nc.gpsimd.dma_start(b2[:], b2_d[:])
            dec_b = wp.tile([1, OP], F32R)
            nc.gpsimd.dma_start(dec_b[:], decb_d[:])

            xaug = sp.tile([KAUG, BL], F32R)
            nc.gpsimd.dma_start(xaug[:], xaug_d[:])
            initt = sp.tile([128, NL * 4 * BL + BL], F32R)
            nc.gpsimd.dma_start(initt[:], init_d[:])
            hT = initt[:, :NL * 4 * BL]
            ones_t = initt[0:1, NL * 4 * BL:NL * 4 * BL + BL]
            ident = sp.tile([BL, BL], F32)
            make_identity(nc, ident)
            cs = []
            for l in range(NL):
                c = sp.tile([BL, H], F32, tag=f"c{l}")
                nc.vector.memset(c[:], 0.0)
                cs.append(c)

            def r(ap):
                return ap

            def hT_sl(l, k):
                j = (l * 4 + k) * BL
                return initt[:, j:j + BL]

            # NCH gate chunks per cell of width GW; narrow (4x512) rotates
            # PSUM slots faster, wide (2x1024) halves matmul issues.
            NCH = globals().get("_NCH", 4)
            GW = G // NCH
            NB = GW * 4 // 2048  # banks per chunk

            def gsl(chunks, lo, hi):
                """yield (global_offset, chunk_ap, slice) covering cols [lo, hi)"""
                for j in range(lo // GW, (hi + GW - 1) // GW):
                    a = max(lo, j * GW) - j * GW
                    b = min(hi, (j + 1) * GW) - j * GW
                    yield j * GW, chunks[j], slice(a, b)

            def emit_hh0(t, ns):
                """cell0 hh matmuls (chunks `ns`) into fresh psum chunks."""
                # (name= explicit: list-comp allocation defeats name inference)
                chunks = [gp.tile([BL, GW], F32, tag="g", name=f"g0_{t}_{n}") for n in ns]
                for chunk, n in zip(chunks, ns):
                    nsl = slice(n * GW, (n + 1) * GW)
                    for k in range(4):
                        nc.tensor.matmul(chunk[:], r(hT_sl(0, k)), r(whh0[:, k, nsl]),
                                         start=(k == 0), stop=False)
                return chunks

            def emit_indep(t, l, wl, bl_t, ns=None):
                chunks = [gp.tile([BL, GW], F32, tag="g", name=f"g{l}_{t}_{n}") for n in (ns or range(NCH))]
                for chunk, n in zip(chunks, ns or range(NCH)):
                    nsl = slice(n * GW, (n + 1) * GW)
                    nc.tensor.matmul(chunk[:], r(ones_t), r(bl_t[:, nsl]),
                                     start=True, stop=False)
                    for k in range(4):
                        nc.tensor.matmul(chunk[:], r(hT_sl(l, k)), r(wl[:, k, nsl]),
                                         start=False, stop=False)
                return chunks

            def emit_inputs(chunks, lsrc, wl):
                for n in range(NCH):
                    nsl = slice(n * GW, (n + 1) * GW)
                    for k in range(4):
                        nc.tensor.matmul(chunks[n][:], r(hT_sl(lsrc, k)), r(wl[:, 4 + k, nsl]),
                                         start=False, stop=(k == 3))

            def lstm_pointwise_transposed(chunks, c, l):
                """Gate nonlinearities + c/h update + h-transposes, half-split
                so the first hT chunks land early for downstream matmuls."""
                ga = hp.tile([BL, G], F32, tag="gact")
                i_ = ga[:, 0 * H:1 * H]
                f_ = ga[:, 1 * H:2 * H]
                g_ = ga[:, 2 * H:3 * H]
                o_ = ga[:, 3 * H:4 * H]
                h = ga[:, 0 * H:1 * H]             # reuse i slot for h
                HH = H // 2
                for off, ch, sl in gsl(chunks, 0, 2 * H):   # sig(i), sig(f)
                    nc.scalar.activation(ga[:, off + sl.start:off + sl.stop], ch[:, sl], ACT.Sigmoid)
                for off, ch, sl in gsl(chunks, 2 * H, 3 * H):
                    nc.scalar.activation(ga[:, off + sl.start:off + sl.stop], ch[:, sl], ACT.Tanh)
                nc.vector.tensor_mul(i_, i_, g_)   # sig(i)*tanh(g)
                nc.vector.tensor_mul(f_, f_, c)    # sig(f)*c
                for off, ch, sl in gsl(chunks, 3 * H, 4 * H):
                    nc.scalar.activation(ga[:, off + sl.start:off + sl.stop], ch[:, sl], ACT.Sigmoid)
                for hf in range(2):
                    sl = slice(hf * HH, (hf + 1) * HH)
                    nc.vector.tensor_add(c[:, sl], i_[:, sl], f_[:, sl])
                    nc.scalar.activation(g_[:, sl], c[:, sl], ACT.Tanh)
                    nc.vector.tensor_mul(h[:, sl], o_[:, sl], g_[:, sl])
                    for k in (2 * hf, 2 * hf + 1):
                        tps = tp.tile([128, BL], F32, tag="tps")
                        nc.tensor.transpose(tps[:], h[:, k * 128:(k + 1) * 128], ident[:])
                        nc.vector.tensor_copy(hT_sl(l, k), tps[:])
                return h

            # prologue: cell0 hh matmuls for t=0
            HALF1 = tuple(range(NCH // 2))
            HALF2 = tuple(range(NCH // 2, NCH))
            g0_chunks = emit_hh0(0, HALF1) + emit_hh0(0, HALF2)
            for t in range(t_steps):
                # (1) cell1 independent: bias + own-h  [dep: hT1(t-1)]
                g1_chunks = emit_indep(t, 1, w1, b1)
                # (2) cell0 aug matmuls  [dep: xaug(t-1 tail)]
                for n in range(NCH):
                    nsl = slice(n * GW, (n + 1) * GW)
                    nc.tensor.matmul(g0_chunks[n][:], r(xaug[:]), r(w0aug[:, nsl]),
                                     start=False, stop=True)
                # (3+5) cell0 pointwise + h0 -> hT0
                lstm_pointwise_transposed(g0_chunks, cs[0], 0)
                # (4a) cell2 independent first half — fills pointwise0
                g2_chunks = emit_indep(t, 2, w2, b2, HALF1)
                # (6) cell1 input matmuls  [dep: hT0(t)]
                emit_inputs(g1_chunks, 0, w1)
                # (4b) cell2 independent second half — fills pointwise1
                g2_chunks = g2_chunks + emit_indep(t, 2, w2, b2, HALF2)
                # (7+9) cell1 pointwise + h1 -> hT1
                lstm_pointwise_transposed(g1_chunks, cs[1], 1)
                # (10) cell2 input matmuls  [dep: hT1(t)]
                emit_inputs(g2_chunks, 1, w2)
                # (11+13) cell2 pointwise + h2 -> hT2
                lstm_pointwise_transposed(g2_chunks, cs[2], 2)
                # (12) next step's cell0 hh (first half) — fills pointwise2
                if t + 1 < t_steps:
                    g0_chunks = emit_hh0(t + 1, HALF1)
                # (14) logits = dec_b + dec @ h2
                lps = tp.tile([BL, OP], F32, tag="tps")
                nc.tensor.matmul(lps[:], r(ones_t), r(dec_b[:]), start=True, stop=False)
                for k in range(4):
                    nc.tensor.matmul(lps[:], r(hT_sl(2, k)), r(dec[:, k, :]),
                                     start=False, stop=(k == 3))
                # (14b) second half of next step's cell0 hh
                if t + 1 < t_steps:
                    g0_chunks = g0_chunks + emit_hh0(t + 1, HALF2)
                # (15) logits tail: out DMA + xaug update
                lt = hp.tile([BL, O], F32, tag="lt")
                nc.vector.tensor_copy(lt[:], lps[:, :O])
                nc.sync.dma_start(out_d[:, t * O:(t + 1) * O], lt[:])
                tps = tp.tile([128, BL], F32, tag="tps")
                nc.tensor.transpose(tps[:O, :], lt[:], ident[:])
                nc.vector.tensor_copy(xaug[0:O, :], tps[:O, :])

    nc.compile()
    return nc


def _init_const():
    init = np.zeros((128, NL * 4 * BL + BL), np.float32)
    init[0, NL * 4 * BL:] = 1.0
    return init


def _host_fold(inputs):
    """Fold encoder/decoder/properties/biases into per-core device inputs."""
    ins = {k: np.asarray(v) for k, v in inputs.items()}
    w_ih0 = ins["w_ih0"].astype(np.float32)
    w_hh0 = ins["w_hh0"].astype(np.float32)
    enc_w = ins["enc_w"].astype(np.float32)
    enc_b = ins["enc_b"].astype(np.float32)
    dec_w = ins["dec_w"].astype(np.float32)
    dec_b = ins["dec_b"].astype(np.float32)
    prop = ins["properties"].astype(np.float32)

    Wx0 = w_ih0[:, :H]
    Wp0 = w_ih0[:, H:]
    A0 = Wx0 @ enc_w                                   # [G, O]
    b0c = Wx0 @ enc_b + ins["b_ih0"] + ins["b_hh0"]    # [G]
    w0aug = np.ascontiguousarray(
        np.concatenate([A0.T, Wp0.T, b0c[None, :].astype(np.float32)], axis=0),
        dtype=np.float32)                              # [52, G]

    def chunked(wT, nk):  # [nk*128, G] -> [128, nk, G]
        return np.ascontiguousarray(
            wT.reshape(nk, 128, wT.shape[1]).transpose(1, 0, 2), dtype=np.float32)

    whh0 = chunked(w_hh0.T, 4)
    W1cat = np.concatenate([ins["w_hh_rest"][0].T, ins["w_ih_rest"][0].T], axis=0)
    W2cat = np.concatenate([ins["w_hh_rest"][1].T, ins["w_ih_rest"][1].T], axis=0)
    w1 = chunked(W1cat.astype(np.float32), 8)
    w2 = chunked(W2cat.astype(np.float32), 8)
    decT_pad = np.zeros((H, OP), np.float32)
    decT_pad[:, :O] = dec_w.T
    dec = chunked(decT_pad, 4)                         # [128, 4, OP]
    b1 = (ins["b_ih_rest"][0] + ins["b_hh_rest"][0]).astype(np.float32)[None, :]
    b2 = (ins["b_ih_rest"][1] + ins["b_hh_rest"][1]).astype(np.float32)[None, :]

    mmdt = np.float16 if MM_DT == "float16" else np.float32
    shared = {
        "w0aug": w0aug.astype(mmdt), "whh0": whh0.astype(mmdt),
        "w1": w1.astype(mmdt), "w2": w2.astype(mmdt), "dec": dec.astype(mmdt),
        "b1": np.ascontiguousarray(b1).astype(mmdt),
        "b2": np.ascontiguousarray(b2).astype(mmdt),
        "dec_b": np.ascontiguousarray(
            np.concatenate([dec_b, np.zeros(OP - O, np.float32)])[None, :]).astype(mmdt),
        "init": _init_const().astype(mmdt),
    }
    in_maps = []
    for cid in range(NCORES):
        xaug = np.zeros((KAUG, BL), np.float32)
        xaug[1, :] = 1.0                               # logits_init = onehot(1)
        xaug[O:O + P, :] = prop[cid * BL:(cid + 1) * BL, :].T
        xaug[O + P, :] = 1.0
        in_maps.append({**shared, "xaug0": np.ascontiguousarray(xaug).astype(mmdt)})
    return in_maps


_NC_CACHE = {}


def _run(inputs, t_steps):
    from concourse.bass_utils import run_bass_kernel_spmd

    if t_steps not in _NC_CACHE:
        _NC_CACHE[t_steps] = _build_nc(t_steps)
    nc = _NC_CACHE[t_steps]
    in_maps = _host_fold(inputs)
    res = run_bass_kernel_spmd(nc, in_maps, core_ids=list(range(NCORES)))
    outs = [res.results[cid]["out"].reshape(BL, t_steps, O) for cid in range(NCORES)]
    return np.concatenate(outs, axis=0)


def kernel(**inputs):
    t_steps = np.asarray(inputs["x"]).shape[1]
    return _run(inputs, t_steps)



# revision 9
# speedup vs baseline: 1.5046x; 1.5046x over previous
"""Trainium2 Bass kernel for a 3-layer conditional LSTM (SMILES RNN) with
encoder/decoder feedback.

v2 design (vs v1 baseline at 1.678ms):
  - Decoder+encoder feedback folded directly into layer-0's recurrence:
      gates0(t) = W0fold @ h2(t-1) + Whh0 @ h0(t-1) + const(props, biases)
    with W0fold = w_ih0[:,:H] @ enc_w @ dec_w (the logits never sit on the
    critical path; they are produced off-path for the output history).
  - Col-tiling: the four 512-wide gate chunks (i,f,g,o) of each layer are
    computed by four concurrent matmul streams into four 32-partition strips
    of ONE PSUM bank (tile_position via out.base_partition()), ~4x the
    weight-stream rate of a single stream.
  - One sigmoid ACT call covers all four gates: tanh(g) = 2*sigmoid(2g)-1
    with the g-rows of every weight/bias scaled x2 host-side; h~ = h/2 =
    (sigmoid(2c)-0.5)*sigmoid(o) with the x2 folded into every h-consuming
    weight matrix.
  - Pointwise is 4 DVE ops/layer via scalar_tensor_tensor fusion:
      m2 = sf*c ; m1 = (sg-0.5)*si ; c' = 2*m1 + m2 ; h~ = (s2c-0.5)*so
  - Per-sample const term (props through w_ih0) added via an identity-
    stationary matmul; L1/L2 biases via ones-row matmuls. All col-tiled.

Distribution: pure data parallel, batch 128 -> 16 rows per core, weights
replicated; the sequential scan stays core-local (no collectives).
"""

import numpy as np

B, T, H, O, P, NL = 128, 64, 512, 47, 4, 3
G = 4 * H
NCORES = 8
BL = B // NCORES
OP = 48  # O padded
GW = 512  # gate chunk width == one gate


def _build_nc(t_steps):
    import concourse.mybir as mybir
    import concourse.tile as tile
    from concourse import bacc
    from concourse.masks import make_identity

    F32 = mybir.dt.float32
    F16 = mybir.dt.float16
    ACT = mybir.ActivationFunctionType
    ALU = mybir.AluOpType

    nc = bacc.Bacc(None, target_bir_lowering=False)

    whh0_d = nc.dram_tensor("whh0", [128, 4, G], F16, kind="ExternalInput")
    w0f_d = nc.dram_tensor("w0f", [128, 4, G], F16, kind="ExternalInput")
    w1_d = nc.dram_tensor("w1", [128, 8, G], F16, kind="ExternalInput")
    w2_d = nc.dram_tensor("w2", [128, 8, G], F16, kind="ExternalInput")
    dec_d = nc.dram_tensor("dec", [128, 4, OP], F16, kind="ExternalInput")
    b1_d = nc.dram_tensor("b1", [1, G], F16, kind="ExternalInput")
    b2_d = nc.dram_tensor("b2", [1, G], F16, kind="ExternalInput")
    decb_d = nc.dram_tensor("dec_b", [1, OP], F16, kind="ExternalInput")
    const_d = nc.dram_tensor("cst", [BL, 2 * G], F16, kind="ExternalInput")
    out_d = nc.dram_tensor("out", [BL, t_steps * O], F32, kind="ExternalOutput")

    with tile.TileContext(nc) as tc:
        with (
            tc.tile_pool(name="weights", bufs=1) as wp,
            tc.tile_pool(name="state", bufs=1) as sp,
            tc.tile_pool(name="work", bufs=2) as hp,
            tc.tile_pool(name="ppool", bufs=1, space="PSUM") as pp,
        ):
            whh0 = wp.tile([128, 4, G], F16)
            nc.gpsimd.dma_start(whh0[:], whh0_d[:])
            w0f = wp.tile([128, 4, G], F16)
            nc.gpsimd.dma_start(w0f[:], w0f_d[:])
            w1 = wp.tile([128, 8, G], F16)
            nc.gpsimd.dma_start(w1[:], w1_d[:])
            w2 = wp.tile([128, 8, G], F16)
            nc.gpsimd.dma_start(w2[:], w2_d[:])
            dec = wp.tile([128, 4, OP], F16)
            nc.gpsimd.dma_start(dec[:], dec_d[:])
            b1 = wp.tile([1, G], F16)
            nc.gpsimd.dma_start(b1[:], b1_d[:])
            b2 = wp.tile([1, G], F16)
            nc.gpsimd.dma_start(b2[:], b2_d[:])
            dec_b = wp.tile([1, OP], F16)
            nc.gpsimd.dma_start(dec_b[:], decb_d[:])
            cst = sp.tile([BL, 2 * G], F16)
            nc.gpsimd.dma_start(cst[:], const_d[:])

            ident16 = sp.tile([BL, BL], F16)
            make_identity(nc, ident16)
            ident112 = sp.tile([112, 112], F16)
            make_identity(nc, ident112)
            ones_t = sp.tile([1, BL], F16)
            nc.vector.memset(ones_t[:], 1.0)

            hT = sp.tile([128, NL * 4, BL], F16)
            nc.vector.memset(hT[:], 0.0)
            gbanks = []
            tbanks = []
            for l in range(NL):
                gb = pp.tile([128, GW], F32, name=f"gbank{l}")
                nc.vector.memset(gb[:], 0.0)
                gbanks.append(gb)
                tb = pp.tile([128, 8, BL], F16, name=f"tbank{l}")
                tbanks.append(tb)
            ps_dec = pp.tile([BL, OP], F32, name="decbank")
            hist = sp.tile([BL, t_steps, OP], F32)
            cs = []
            for l in range(NL):
                c = sp.tile([112, 128], F32, tag=f"c{l}")
                nc.vector.memset(c[:], 0.0)
                cs.append(c)

            def hT_sl(l, k):
                j = l * 4 + k
                return hT[:, j:j + 1, :]

            def emit_bias_hh(t, gl, l, wl, bl_t):
                """L1/L2 independent part: bias + own-h.  k-outer emission so
                each round of 4 MMs streams concurrently in 4 col groups."""
                for j in range(4):
                    nc.tensor.matmul(gl[32 * j:32 * j + BL, :], ones_t[:],
                                     bl_t[:, j * GW:(j + 1) * GW], start=True,
                                     stop=False, tile_position=(0, 32 * j),
                                     skip_group_check=True)
                for k in range(4):
                    for j in range(4):
                        nc.tensor.matmul(gl[32 * j:32 * j + BL, :], hT_sl(l, k),
                                         wl[:, k, j * GW:(j + 1) * GW],
                                         start=False, stop=False,
                                         tile_position=(0, 32 * j),
                                         skip_group_check=True)

            def emit_ih(t, gl, lsrc, wl):
                """L1/L2 input part from layer lsrc's fresh h; closes group."""
                for k in range(4):
                    for j in range(4):
                        nc.tensor.matmul(gl[32 * j:32 * j + BL, :], hT_sl(lsrc, k),
                                         wl[:, 4 + k, j * GW:(j + 1) * GW],
                                         start=False, stop=(k == 3),
                                         tile_position=(0, 32 * j),
                                         skip_group_check=True)

            def emit_const_hh0(t, gl):
                """L0 independent part: const(props,biases,dec_b) + own-h."""
                csl = cst[:, 0:G] if t == 0 else cst[:, G:2 * G]
                for j in range(4):
                    nc.tensor.matmul(gl[32 * j:32 * j + BL, :], ident16[:],
                                     csl[:, j * GW:(j + 1) * GW], start=True,
                                     stop=False, tile_position=(0, 32 * j),
                                     skip_group_check=True)
                for k in range(4):
                    for j in range(4):
                        nc.tensor.matmul(gl[32 * j:32 * j + BL, :], hT_sl(0, k),
                                         whh0[:, k, j * GW:(j + 1) * GW],
                                         start=False, stop=False,
                                         tile_position=(0, 32 * j),
                                         skip_group_check=True)

            def emit_w0f(t, gl):
                """L0 folded decoder->input part from h2; closes group."""
                for k in range(4):
                    for j in range(4):
                        nc.tensor.matmul(gl[32 * j:32 * j + BL, :], hT_sl(2, k),
                                         w0f[:, k, j * GW:(j + 1) * GW],
                                         start=False, stop=(k == 3),
                                         tile_position=(0, 32 * j),
                                         skip_group_check=True)

            def pointwise(gl, l, t):
                """gates: strip q (partitions 32q..32q+16) holds quarter q of
                all four gates as [i.q|f.q|g.q|o.q] x 128 cols each.  All
                pointwise ops are strip-aligned: 112 partitions, FD=128."""
                ga = hp.tile([128, GW], F16, tag="ga", name=f"ga{l}_{t}")
                nc.scalar.activation(ga[0:112, :], gl[0:112, :], ACT.Sigmoid)
                si = ga[0:112, 0:128]
                sf = ga[0:112, 128:256]
                sg = ga[0:112, 256:384]
                so = ga[0:112, 384:512]
                c = cs[l]
                m2 = hp.tile([112, 128], F32, tag="m2", name=f"m2_{l}_{t}")
                nc.vector.tensor_mul(m2[:], sf, c[:])
                m1 = hp.tile([112, 128], F32, tag="m1", name=f"m1_{l}_{t}")
                nc.vector.scalar_tensor_tensor(
                    m1[:], sg, 0.5, si, op0=ALU.subtract, op1=ALU.mult)
                nc.vector.scalar_tensor_tensor(
                    c[:], m1[:], 2.0, m2[:], op0=ALU.mult, op1=ALU.add)
                s2 = hp.tile([112, 128], F16, tag="s2", name=f"s2_{l}_{t}")
                nc.scalar.activation(s2[:], c[:], ACT.Sigmoid, scale=2.0)
                h = hp.tile([112, 128], F16, tag="h", name=f"h_{l}_{t}")
                nc.vector.scalar_tensor_tensor(
                    h[:], s2[:], 0.5, so, op0=ALU.subtract, op1=ALU.mult)
                tps = tbanks[l]
                nc.tensor.transpose(tps[:, 0:7, :], h[:], ident112[:])
                nc.vector.tensor_copy(hT[:, l * 4:(l + 1) * 4, :], tps[:, 0:8:2, :])

            def emit_dec(t):
                nc.tensor.matmul(ps_dec[:], ones_t[:], dec_b[:], start=True, stop=False)
                for k in range(4):
                    nc.tensor.matmul(ps_dec[:], hT_sl(2, k), dec[:, k, :],
                                     start=False, stop=(k == 3))
                nc.vector.tensor_copy(hist[:, t, :], ps_dec[:])

            # ---- prologue: step 0 L0 gates (h=0 terms included uniformly)
            emit_const_hh0(0, gbanks[0])
            emit_w0f(0, gbanks[0])
            emit_bias_hh(0, gbanks[1], 1, w1, b1)

            for t in range(t_steps):
                # PW0 -> h0, h0T
                pointwise(gbanks[0], 0, t)
                # L1 input matmuls (critical tail for PW1)
                emit_ih(t, gbanks[1], 0, w1)
                # L2 independent part (runs during PW1)
                emit_bias_hh(t, gbanks[2], 2, w2, b2)
                # PW1 -> h1, h1T
                pointwise(gbanks[1], 1, t)
                # L2 input matmuls (critical tail for PW2)
                emit_ih(t, gbanks[2], 1, w2)
                # next step L0 independent part (runs during PW2)
                if t + 1 < t_steps:
                    emit_const_hh0(t + 1, gbanks[0])
                # PW2 -> h2, h2T
                pointwise(gbanks[2], 2, t)
                # next step L0 folded-decoder part (critical tail for PW0')
                if t + 1 < t_steps:
                    emit_w0f(t + 1, gbanks[0])
                # logits for the output history (off critical path)
                emit_dec(t)
                # next step L1 independent part (runs during PW0')
                if t + 1 < t_steps:
                    emit_bias_hh(t + 1, gbanks[1], 1, w1, b1)

            nc.sync.dma_start(out_d[:], hist[:, :, 0:O])

    nc.compile()
    return nc


def _host_fold(inputs):
    """Fold encoder/decoder/properties/biases; scale g-rows x2 (tanh via
    sigmoid) and every h-consuming weight x2 (h~ = h/2 on device)."""
    ins = {k: np.asarray(v) for k, v in inputs.items()}
    f64 = np.float64
    w_ih0 = ins["w_ih0"].astype(f64)
    w_hh0 = ins["w_hh0"].astype(f64)
    enc_w = ins["enc_w"].astype(f64)
    enc_b = ins["enc_b"].astype(f64)
    dec_w = ins["dec_w"].astype(f64)
    dec_b = ins["dec_b"].astype(f64)
    prop = ins["properties"].astype(f64)

    gscale = np.ones((G,), f64)
    gscale[2 * H:3 * H] = 2.0

    Wx0 = w_ih0[:, :H]
    Wp0 = w_ih0[:, H:]
    A0 = Wx0 @ enc_w                                    # [G, O]
    W0f_full = 2.0 * (A0 @ dec_w) * gscale[:, None]     # [G, H]
    Whh0_full = 2.0 * w_hh0 * gscale[:, None]           # [G, H]

    const_common = Wx0 @ enc_b + ins["b_ih0"].astype(f64) + ins["b_hh0"].astype(f64)
    const_t1 = prop @ Wp0.T + const_common + A0 @ dec_b   # [B, G]
    const_t0 = prop @ Wp0.T + const_common + A0[:, 1]     # [B, G]
    const_t0 = const_t0 * gscale
    const_t1 = const_t1 * gscale

    W1_full = 2.0 * np.concatenate(
        [ins["w_hh_rest"][0], ins["w_ih_rest"][0]], axis=1).astype(f64) * gscale[:, None]
    W2_full = 2.0 * np.concatenate(
        [ins["w_hh_rest"][1], ins["w_ih_rest"][1]], axis=1).astype(f64) * gscale[:, None]
    b1 = (ins["b_ih_rest"][0] + ins["b_hh_rest"][0]).astype(f64) * gscale
    b2 = (ins["b_ih_rest"][1] + ins["b_hh_rest"][1]).astype(f64) * gscale
    dec_full = 2.0 * dec_w                               # [O, H]

    # Quarter-strip gate-column permutation: chunk j = [i.qj|f.qj|g.qj|o.qj]
    perm = np.concatenate(
        [np.arange(gate * 512 + 128 * j, gate * 512 + 128 * (j + 1))
         for j in range(4) for gate in range(4)])

    def chunked(wT, nk):  # [nk*128, G or OP] -> [128, nk, *]
        return np.ascontiguousarray(
            wT.reshape(nk, 128, wT.shape[1]).transpose(1, 0, 2)).astype(np.float16)

    decT_pad = np.zeros((H, OP), f64)
    decT_pad[:, :O] = dec_full.T
    shared = {
        "whh0": chunked(Whh0_full.T[:, perm], 4),
        "w0f": chunked(W0f_full.T[:, perm], 4),
        "w1": chunked(W1_full.T[:, perm], 8),
        "w2": chunked(W2_full.T[:, perm], 8),
        "dec": chunked(decT_pad, 4),
        "b1": np.ascontiguousarray(b1[None, perm]).astype(np.float16),
        "b2": np.ascontiguousarray(b2[None, perm]).astype(np.float16),
        "dec_b": np.ascontiguousarray(
            np.concatenate([dec_b, np.zeros(OP - O)])[None, :]).astype(np.float16),
    }
    in_maps = []
    for cid in range(NCORES):
        rows = slice(cid * BL, (cid + 1) * BL)
        cst = np.concatenate([const_t0[rows][:, perm], const_t1[rows][:, perm]], axis=1)
        in_maps.append(
            {**shared, "cst": np.ascontiguousarray(cst).astype(np.float16)})
    return in_maps


_NC_CACHE = {}


def _run(inputs, t_steps):
    import os
    from concourse.bass_utils import run_bass_kernel_spmd

    if t_steps not in _NC_CACHE:
        _NC_CACHE[t_steps] = _build_nc(t_steps)
    nc = _NC_CACHE[t_steps]
    in_maps = _host_fold(inputs)
    res = run_bass_kernel_spmd(nc, in_maps, core_ids=list(range(NCORES)))
    if getattr(res, "exec_time_ns", None):
        print(f"[kernel] device exec_time_ns: {res.exec_time_ns}")
    outs = [res.results[cid]["out"].reshape(BL, t_steps, O) for cid in range(NCORES)]
    return np.concatenate(outs, axis=0).astype(np.float32)


def kernel(**inputs):
    t_steps = np.asarray(inputs["x"]).shape[1]
    return _run(inputs, t_steps)
